# revision 9
# baseline (speedup 1.0000x reference)
"""BitNetV3 transformer block on 8 Trainium2 NeuronCores.

Sharding: sequence-parallel. Each core owns 512 query tokens (two
256-token blocks (g, g+4) of one batch element; cores 0-3 -> batch 0,
cores 4-7 -> batch 1). Weights are replicated, host-pre-transposed and
bf16-cast so every matmul's stationary operand DMAs naturally. K/V are
computed per-core for owned tokens and exchanged with two 4-rank
AllGathers (replica groups {0-3}, {4-7}). Causal masking uses
host-supplied per-core 0/1 mask tiles so the SPMD program is identical
on every core. Activations live transposed ([d, token]) end to end;
per-token reductions (rmsnorm stats, softmax denominators) use
ones-vector matmuls onto partition 0 + gpsimd partition_broadcast.

Each core returns its tokens' [128, 16, 512] slice; the host
reassembles the full (2, 2048, 2048) output.
"""

import os
from contextlib import ExitStack

import numpy as np
import ml_dtypes

# ---- problem constants (hardcoded per the harness contract) ----
B = 2
S = 2048
D = 2048
H = 16
HD = 128
DFF = 8192
EPS = 1e-6
ISQ = float(1.0 / np.sqrt(HD))

P = 128  # partitions
KO = D // P  # 16 d-tiles
Q = 512  # tokens per core
NB = S // P  # 16 k-tiles per batch
MF = DFF // P  # 64 dff-tiles
BLK = 256  # token block
NCORES = 8

BF16 = ml_dtypes.bfloat16


# ---------------------------------------------------------------------------
# Host-side data preparation (sharding + layout)
# ---------------------------------------------------------------------------

def _w5(w_t: np.ndarray, kt: int, mt: int) -> np.ndarray:
    """[K, M] (transposed weight, contraction-major) -> [128, mt, kt, 128]
    with W5[p, m, k, i] = w_t[k*128+p, m*128+i]."""
    K, M = w_t.shape
    assert K == kt * P and M == mt * P
    return np.ascontiguousarray(
        w_t.reshape(kt, P, mt, P).transpose(1, 2, 0, 3)
    ).astype(BF16)


def _core_tokens(g: int) -> np.ndarray:
    """Global (within-batch) token indices for group-rank g: blocks g, g+4."""
    t1 = np.arange(BLK * g, BLK * (g + 1))
    t2 = np.arange(BLK * (g + 4), BLK * (g + 5))
    return np.concatenate([t1, t2])


def _core_mask(g: int) -> np.ndarray:
    """[128, NB, 512] bf16 causal mask in the device layout.

    jj < 8: cols 0..511 = all 512 owned q tokens.
    jj >= 8: cols 0..255 = q tokens of block g+4 (shifted layout)."""
    toks = _core_tokens(g)  # 512 global q indices
    m = np.zeros((P, NB, Q), dtype=np.float32)
    for jj in range(NB):
        kk = 128 * jj + np.arange(P)  # global k indices of this tile
        if jj < 8:
            m[:, jj, :] = kk[:, None] <= toks[None, :]
        else:
            m[:, jj, :BLK] = kk[:, None] <= toks[None, BLK:]
    return m.astype(BF16)


def prepare_shared(wq, wk, wv, wo, w_gate, w_up, w_down, ln1_w, ln2_w):
    d = {}
    d["wqt"] = _w5(np.ascontiguousarray(wq.T), KO, KO)
    d["wkt"] = _w5(np.ascontiguousarray(wk.T), KO, KO)
    d["wot"] = _w5(np.ascontiguousarray(wo.T), KO, KO)
    # V projection rhs layout: [128, ko, 2048]
    d["wvt"] = np.ascontiguousarray(
        wv.T.reshape(KO, P, D).transpose(1, 0, 2)
    ).astype(BF16)
    d["wgt"] = _w5(np.ascontiguousarray(w_gate.T), KO, MF)
    d["wut"] = _w5(np.ascontiguousarray(w_up.T), KO, MF)
    d["wdt"] = _w5(np.ascontiguousarray(w_down.T), MF, KO)
    d["ln1"] = np.ascontiguousarray(ln1_w.reshape(KO, P).T).astype(np.float32)
    d["ln2"] = np.ascontiguousarray(ln2_w.reshape(KO, P).T).astype(np.float32)
    return d


def prepare_core(hidden, core: int):
    b, g = core // 4, core % 4
    toks = _core_tokens(g)
    ht = hidden[b][toks].T  # [2048 d, 512 q]
    ht5 = np.ascontiguousarray(ht.reshape(KO, P, Q).transpose(1, 0, 2)).astype(
        np.float32
    )
    return {"ht": ht5, "mask": _core_mask(g)}


def assemble(outs, hidden_dtype):
    """outs: list of per-core [128, KO, 512] fp32 -> full (B, S, D)."""
    full = np.empty((B, S, D), dtype=np.float32)
    for core in range(NCORES):
        b, g = core // 4, core % 4
        toks = _core_tokens(g)
        o = np.asarray(outs[core])  # [p, ko, q]
        full[b, toks, :] = o.transpose(2, 1, 0).reshape(Q, D)
    return full.astype(hidden_dtype)


def _agcol(jj: int) -> int:
    """Global k-tile jj -> row/col offset in the rank-major AG buffers."""
    j = jj // 2  # 256-token block index
    return 512 * (j % 4) + 256 * (j // 4) + 128 * (jj % 2)


# ---------------------------------------------------------------------------
# Pure-numpy simulation of the exact device dataflow (for fast validation)
# ---------------------------------------------------------------------------

def _bf(x):
    return x.astype(BF16).astype(np.float32)


def _sim_norm(ht, lnw):
    # ht: [2048, 512] fp32 (d, q); lnw: [2048]
    sq = _bf(_bf(ht) * _bf(ht))  # DVE squares, bf16 out
    ms = sq.sum(axis=0)  # PE ones-matmul, fp32 accum
    rstd = 1.0 / np.sqrt(ms / D + EPS)
    return _bf((ht * rstd[None, :]) * lnw[:, None])  # bf16 out


def host_simulate(inputs):
    """Numpy replica of the device algorithm, including AG layout and masks."""
    hidden = np.asarray(inputs["hidden_states"], dtype=np.float32)
    f32 = lambda k: np.asarray(inputs[k], dtype=np.float32)  # noqa: E731
    wqT, wkT, wvT, woT = (
        _bf(f32("wq").T), _bf(f32("wk").T), _bf(f32("wv").T), _bf(f32("wo").T)
    )
    wgT, wuT, wdT = _bf(f32("w_gate").T), _bf(f32("w_up").T), _bf(f32("w_down").T)
    ln1, ln2 = f32("ln1_w"), f32("ln2_w")

    kts, vs, xns, hts = {}, {}, {}, {}
    for core in range(NCORES):
        b, g = core // 4, core % 4
        ht = hidden[b][_core_tokens(g)].T  # [2048, 512]
        hts[core] = ht
        xn = _sim_norm(ht, ln1)
        xns[core] = xn
        kts[core] = _bf(wkT.T @ xn)  # kT [2048, 512]
        vs[core] = _bf(xn.T @ wvT)  # v natural [512, 2048]

    outs = []
    for core in range(NCORES):
        b, g = core // 4, core % 4
        grp = [4 * b + r for r in range(4)]
        kt_all = np.concatenate([kts[c] for c in grp], axis=1)  # [2048, 2048]
        v_all = np.concatenate([vs[c] for c in grp], axis=0)  # [2048, 2048]
        mask = np.asarray(_core_mask(g), dtype=np.float32)

        xn = xns[core]
        qT = _bf(wqT.T @ xn)  # [2048, 512]
        attn = np.zeros((D, Q), dtype=np.float32)
        for h in range(H):
            kth = kt_all[h * HD : (h + 1) * HD]  # [128, 2048]
            aps = np.zeros((HD, Q), dtype=np.float32)
            den = np.zeros(Q, dtype=np.float32)
            for jj in range(NB):
                off = _agcol(jj)
                n = Q if jj < 8 else BLK
                sc = kth[:, off : off + P].T @ qT[h * HD : (h + 1) * HD, Q - n :]
                e = _bf(_bf(np.exp(sc * ISQ)) * mask[:, jj, :n])
                vt = v_all[off : off + P, h * HD : (h + 1) * HD]  # [128, hd]
                aps[:, Q - n :] += vt.T @ e
                den[Q - n :] += e.sum(axis=0)
            attn[h * HD : (h + 1) * HD] = _bf(aps * (1.0 / den)[None, :])
        oT = woT.T @ attn  # fp32 accum of bf16 matmul
        h2 = hts[core] + oT
        yT = _sim_norm(h2, ln2)
        gate = wgT.T @ yT
        up = wuT.T @ yT
        sil = _bf(gate / (1.0 + np.exp(-gate)))
        hmid = _bf(sil * up)
        outT = h2 + wdT.T @ hmid
        outs.append(outT.reshape(KO, P, Q).transpose(1, 0, 2).astype(np.float32))
    return assemble(outs, np.asarray(inputs["hidden_states"]).dtype)


# ---------------------------------------------------------------------------
# Device program
# ---------------------------------------------------------------------------

def _build_bass():
    import concourse.bacc as bacc
    import concourse.mybir as mybir
    import concourse.tile as tile

    FP = mybir.dt.float32
    BF = mybir.dt.bfloat16
    AF = mybir.ActivationFunctionType

    nc = bacc.Bacc("TRN2", target_bir_lowering=False, debug=False,
                   num_devices=NCORES)

    ht_d = nc.dram_tensor("ht", [P, KO, Q], FP, kind="ExternalInput")
    mask_d = nc.dram_tensor("mask", [P, NB, Q], BF, kind="ExternalInput")
    ln1_d = nc.dram_tensor("ln1", [P, KO], FP, kind="ExternalInput")
    ln2_d = nc.dram_tensor("ln2", [P, KO], FP, kind="ExternalInput")
    wqt_d = nc.dram_tensor("wqt", [P, KO, KO, P], BF, kind="ExternalInput")
    wkt_d = nc.dram_tensor("wkt", [P, KO, KO, P], BF, kind="ExternalInput")
    wvt_d = nc.dram_tensor("wvt", [P, KO, D], BF, kind="ExternalInput")
    wot_d = nc.dram_tensor("wot", [P, KO, KO, P], BF, kind="ExternalInput")
    wgt_d = nc.dram_tensor("wgt", [P, MF, KO, P], BF, kind="ExternalInput")
    wut_d = nc.dram_tensor("wut", [P, MF, KO, P], BF, kind="ExternalInput")
    wdt_d = nc.dram_tensor("wdt", [P, KO, MF, P], BF, kind="ExternalInput")
    out_d = nc.dram_tensor("out", [P, KO, Q], FP, kind="ExternalOutput")

    groups = [[0, 1, 2, 3], [4, 5, 6, 7]]

    with tile.TileContext(nc) as tc, ExitStack() as top:
        dramp = top.enter_context(tc.tile_pool(name="dram", bufs=1, space="DRAM"))
        constp = top.enter_context(tc.tile_pool(name="const", bufs=1))
        statp = top.enter_context(tc.tile_pool(name="stat", bufs=2))
        workp = top.enter_context(tc.tile_pool(name="work", bufs=3))
        psump = top.enter_context(tc.tile_pool(name="ps", bufs=4, space="PSUM"))
        psaccp = psump

        ones = constp.tile([P, P], BF, tag="ones")
        nc.vector.memset(ones, 1.0)
        eps_t = constp.tile([P, 1], FP, tag="eps")
        nc.vector.memset(eps_t, EPS)
        lnw1 = constp.tile([P, KO], FP, tag="ln1")
        nc.sync.dma_start(lnw1, ln1_d[:])
        lnw2 = constp.tile([P, KO], FP, tag="ln2")
        nc.sync.dma_start(lnw2, ln2_d[:])

        ht_sb = constp.tile([P, KO, Q], FP, tag="ht")  # becomes h2 in place
        nc.sync.dma_start(ht_sb, ht_d[:])
        xn_sb = constp.tile([P, KO, Q], BF, tag="xn")  # x_norm^T, later y^T

        kv_bounce = dramp.tile([2, D * Q], BF)
        kv_all = dramp.tile([8, D * Q], BF)
        kt_bounce = kv_bounce[0:1, :].rearrange("a (d q) -> (a d) q", q=Q)
        v_bounce = kv_bounce[1:2, :].rearrange("a (t d) -> (a t) d", d=D)

        def norm(src, lnw, dst):
            ms = psaccp.tile([P, Q], FP, tag="acc")
            for ko in range(KO):
                sq = workp.tile([P, Q], BF, tag="sq")
                nc.vector.tensor_mul(sq, src[:, ko, :], src[:, ko, :])
                nc.tensor.matmul(ms, ones, sq, start=(ko == 0),
                                 stop=(ko == KO - 1))
            st = statp.tile([P, Q], FP, tag="st")
            nc.scalar.activation(st, ms, AF.Sqrt, bias=eps_t,
                                 scale=1.0 / D)
            rb = statp.tile([P, Q], FP, tag="rb")
            nc.vector.reciprocal(rb, st)
            for ko in range(KO):
                tmp = workp.tile([P, Q], FP, tag="nrm")
                nc.vector.tensor_mul(tmp, src[:, ko, :], rb)
                nc.vector.tensor_scalar_mul(dst[:, ko, :], tmp,
                                            lnw[:, ko : ko + 1])

        # ---- phase 1: norm1 ----
        norm(ht_sb, lnw1, xn_sb)

        with ExitStack() as mid:
            midp = mid.enter_context(tc.tile_pool(name="mid", bufs=1))
            qt_sb = midp.tile([P, KO, Q], BF, tag="qt")
            attn_sb = midp.tile([P, KO, Q], BF, tag="attn")
            with ExitStack() as ctx:
                wp = ctx.enter_context(tc.tile_pool(name="wqkv", bufs=3))
                wvp = ctx.enter_context(tc.tile_pool(name="wvp", bufs=2))

                def proj_t(w5_d, dst_fn):
                    for mt in range(KO):
                        wt = wp.tile([P, KO, P], BF, tag="wqk")
                        nc.sync.dma_start(wt, w5_d[:, mt, :, :])
                        ps = psump.tile([P, Q], FP, tag="mm")
                        for ko in range(KO):
                            nc.tensor.matmul(ps, wt[:, ko, :], xn_sb[:, ko, :],
                                             start=(ko == 0),
                                             stop=(ko == KO - 1))
                        dst_fn(mt, ps)

                # ---- phase 2: K projection + AG ----
                def k_out(mt, ps):
                    stg = workp.tile([P, Q], BF, tag="stg")
                    nc.vector.tensor_copy(stg, ps)
                    nc.sync.dma_start(
                        kt_bounce[mt * P : (mt + 1) * P, :], stg
                    )

                proj_t(wkt_d, k_out)

                # ---- phase 3: V projection + AG ----
                for n in range(4):
                    wv_c = wvp.tile([P, KO, Q], BF, tag="wv")
                    nc.sync.dma_start(wv_c, wvt_d[:, :, n * Q : (n + 1) * Q])
                    for tt in range(4):
                        ps = psump.tile([P, Q], FP, tag="mm")
                        for ko in range(KO):
                            nc.tensor.matmul(
                                ps, xn_sb[:, ko, tt * P : (tt + 1) * P],
                                wv_c[:, ko, :],
                                start=(ko == 0), stop=(ko == KO - 1),
                            )
                        stg = workp.tile([P, Q], BF, tag="stg")
                        nc.vector.tensor_copy(stg, ps)
                        nc.sync.dma_start(
                            v_bounce[tt * P : (tt + 1) * P,
                                     n * Q : (n + 1) * Q],
                            stg,
                        )
                nc.gpsimd.collective_compute(
                    "AllGather", mybir.AluOpType.bypass,
                    ins=[kv_bounce.opt()], outs=[kv_all.opt()],
                    replica_groups=groups,
                )

                # ---- phase 4: Q projection ----
                proj_t(wqt_d, lambda mt, ps:
                       nc.vector.tensor_copy(qt_sb[:, mt, :], ps))

            # ---- phase 5: attention ----
            with ExitStack() as ctx:
                ap = ctx.enter_context(tc.tile_pool(name="attp", bufs=3))
                eap = ctx.enter_context(tc.tile_pool(name="eap", bufs=2))
                maskp = ctx.enter_context(tc.tile_pool(name="maskp", bufs=1))
                mask_sb = maskp.tile([P, NB, Q], BF, tag="mask")
                nc.sync.dma_start(mask_sb, mask_d[:])
                kt_all_r = kv_all.rearrange(
                    "(r a) (p q) -> a r p q", a=2, q=Q
                )[0].rearrange("r (m p) q -> p r m q", p=P)
                v_all_r = kv_all.rearrange(
                    "(r a) (t d) -> a r t d", a=2, d=D
                )[1]

                for h in range(H):
                    kth = ap.tile([P, 4, Q], BF, tag="kth")
                    nc.sync.dma_start(kth, kt_all_r[:, :, h, :])
                    kth2 = kth.rearrange("p r q -> p (r q)")
                    vth = ap.tile([P, NB, HD], BF, tag="vth")
                    for jj in range(NB):
                        off = _agcol(jj)
                        nc.sync.dma_start(
                            vth[:, jj, :],
                            v_all_r[off // Q, off % Q : off % Q + P,
                                    h * HD : (h + 1) * HD],
                        )
                    e_all = eap.tile([P, NB, Q], BF, tag="eall")
                    # pass A: scores + exp + mask (needs only KT)
                    for jj in range(NB):
                        n = Q if jj < 8 else BLK
                        off = _agcol(jj)
                        sps = psump.tile([P, Q], FP, tag="mm")
                        nc.tensor.matmul(
                            sps[:, :n], kth2[:, off : off + P],
                            qt_sb[:, h, Q - n :], start=True, stop=True,
                        )
                        nc.scalar.activation(e_all[:, jj, :n], sps[:, :n],
                                             AF.Exp, scale=ISQ)
                        nc.vector.tensor_mul(e_all[:, jj, :n],
                                             e_all[:, jj, :n],
                                             mask_sb[:, jj, :n])
                    # pass B: PV + denominator (needs V)
                    aps = psaccp.tile([P, Q], FP, tag="acc")
                    dps = psaccp.tile([P, Q], FP, tag="acc")
                    for jj in range(NB):
                        n = Q if jj < 8 else BLK
                        osl = slice(Q - n, Q)
                        nc.tensor.matmul(aps[:, osl], vth[:, jj, :],
                                         e_all[:, jj, :n],
                                         start=(jj == 0), stop=(jj == NB - 1))
                        nc.tensor.matmul(dps[:, osl], ones,
                                         e_all[:, jj, :n],
                                         start=(jj == 0), stop=(jj == NB - 1))
                    rec = statp.tile([P, Q], FP, tag="rb")
                    nc.vector.reciprocal(rec, dps)
                    nc.vector.tensor_mul(attn_sb[:, h, :], aps, rec)

            # ---- phase 6: o-projection + residual (into ht_sb) ----
            with ExitStack() as ctx:
                wp = ctx.enter_context(tc.tile_pool(name="wo", bufs=3))
                for mt in range(KO):
                    wt = wp.tile([P, KO, P], BF, tag="wqk")
                    nc.sync.dma_start(wt, wot_d[:, mt, :, :])
                    ps = psump.tile([P, Q], FP, tag="mm")
                    for ko in range(KO):
                        nc.tensor.matmul(ps, wt[:, ko, :], attn_sb[:, ko, :],
                                         start=(ko == 0), stop=(ko == KO - 1))
                    nc.vector.tensor_add(ht_sb[:, mt, :], ps, ht_sb[:, mt, :])

        # ---- phase 7: norm2 (into xn_sb = y^T) ----
        norm(ht_sb, lnw2, xn_sb)

        # ---- phases 8+9: MLP ----
        with ExitStack() as ctx:
            wgp = ctx.enter_context(tc.tile_pool(name="wgu", bufs=2))
            wdp = ctx.enter_context(tc.tile_pool(name="wdp", bufs=2))
            hp = ctx.enter_context(tc.tile_pool(name="hmid", bufs=1))
            hmid = hp.tile([P, MF, Q], BF, tag="hmid")
            for mf in range(MF):
                wg_t = wgp.tile([P, KO, P], BF, tag="wg")
                nc.sync.dma_start(wg_t, wgt_d[:, mf, :, :])
                wu_t = wgp.tile([P, KO, P], BF, tag="wu")
                nc.sync.dma_start(wu_t, wut_d[:, mf, :, :])
                gps = psump.tile([P, Q], FP, tag="mm")
                ups = psump.tile([P, Q], FP, tag="mm")
                for ko in range(KO):
                    nc.tensor.matmul(gps, wg_t[:, ko, :], xn_sb[:, ko, :],
                                     start=(ko == 0), stop=(ko == KO - 1))
                for ko in range(KO):
                    nc.tensor.matmul(ups, wu_t[:, ko, :], xn_sb[:, ko, :],
                                     start=(ko == 0), stop=(ko == KO - 1))
                sil = workp.tile([P, Q], BF, tag="sil")
                nc.scalar.activation(sil, gps, AF.Silu)
                nc.vector.tensor_mul(hmid[:, mf, :], sil, ups)

            for mt in range(KO):
                wd_t = wdp.tile([P, MF, P], BF, tag="wd")
                nc.sync.dma_start(wd_t, wdt_d[:, mt, :, :])
                ps = psump.tile([P, Q], FP, tag="mm")
                for kf in range(MF):
                    nc.tensor.matmul(ps, wd_t[:, kf, :], hmid[:, kf, :],
                                     start=(kf == 0), stop=(kf == MF - 1))
                ot = workp.tile([P, Q], FP, tag="ot")
                nc.vector.tensor_add(ot, ps, ht_sb[:, mt, :])
                nc.sync.dma_start(out_d[:, mt, :], ot)

    nc.compile()
    return nc


_NC_CACHE = None


def kernel(**inputs) -> np.ndarray:
    global _NC_CACHE
    hidden = np.asarray(inputs["hidden_states"])
    shared = prepare_shared(
        np.asarray(inputs["wq"]), np.asarray(inputs["wk"]),
        np.asarray(inputs["wv"]), np.asarray(inputs["wo"]),
        np.asarray(inputs["w_gate"]), np.asarray(inputs["w_up"]),
        np.asarray(inputs["w_down"]), np.asarray(inputs["ln1_w"]),
        np.asarray(inputs["ln2_w"]),
    )
    in_maps = []
    for core in range(NCORES):
        m = dict(shared)
        m.update(prepare_core(np.asarray(hidden, dtype=np.float32), core))
        in_maps.append(m)

    from concourse.bass_utils import run_bass_kernel_spmd

    if _NC_CACHE is None:
        _NC_CACHE = _build_bass()
    nc = _NC_CACHE
    trace = bool(int(os.environ.get("KERNEL_TRACE", "0")))
    res = run_bass_kernel_spmd(
        nc, in_maps, core_ids=list(range(NCORES)), trace=trace
    )
    if trace and res.exec_time_ns is not None:
        print(f"HW exec time: {res.exec_time_ns} ns")
    outs = [res.results[c]["out"] for c in range(NCORES)]
    return assemble(outs, hidden.dtype)


# revision 11
# speedup vs baseline: 1.0324x; 1.0324x over previous
"""BitNetV3 transformer block on 8 Trainium2 NeuronCores.

Sharding: sequence-parallel. Each core owns 512 query tokens (two
256-token blocks (g, g+4) of one batch element; cores 0-3 -> batch 0,
cores 4-7 -> batch 1). Weights are replicated, host-pre-transposed and
bf16-cast so every matmul's stationary operand DMAs naturally. K/V are
computed per-core for owned tokens and exchanged with two 4-rank
AllGathers (replica groups {0-3}, {4-7}). Causal masking uses
host-supplied per-core 0/1 mask tiles so the SPMD program is identical
on every core. Activations live transposed ([d, token]) end to end;
per-token reductions (rmsnorm stats, softmax denominators) use
ones-vector matmuls onto partition 0 + gpsimd partition_broadcast.

Each core returns its tokens' [128, 16, 512] slice; the host
reassembles the full (2, 2048, 2048) output.
"""

import os
from contextlib import ExitStack

import numpy as np
import ml_dtypes

# ---- problem constants (hardcoded per the harness contract) ----
B = 2
S = 2048
D = 2048
H = 16
HD = 128
DFF = 8192
EPS = 1e-6
ISQ = float(1.0 / np.sqrt(HD))

P = 128  # partitions
KO = D // P  # 16 d-tiles
Q = 512  # tokens per core
NB = S // P  # 16 k-tiles per batch
MF = DFF // P  # 64 dff-tiles
BLK = 256  # token block
NCORES = 8

BF16 = ml_dtypes.bfloat16


# ---------------------------------------------------------------------------
# Host-side data preparation (sharding + layout)
# ---------------------------------------------------------------------------

def _w5(w_t: np.ndarray, kt: int, mt: int) -> np.ndarray:
    """[K, M] (transposed weight, contraction-major) -> [128, mt, kt, 128]
    with W5[p, m, k, i] = w_t[k*128+p, m*128+i]."""
    K, M = w_t.shape
    assert K == kt * P and M == mt * P
    return np.ascontiguousarray(
        w_t.reshape(kt, P, mt, P).transpose(1, 2, 0, 3)
    ).astype(BF16)


def _core_tokens(g: int) -> np.ndarray:
    """Global (within-batch) token indices for group-rank g: blocks g, g+4."""
    t1 = np.arange(BLK * g, BLK * (g + 1))
    t2 = np.arange(BLK * (g + 4), BLK * (g + 5))
    return np.concatenate([t1, t2])


def _core_mask(g: int) -> np.ndarray:
    """[128, NB, 512] bf16 causal mask in the device layout.

    jj < 8: cols 0..511 = all 512 owned q tokens.
    jj >= 8: cols 0..255 = q tokens of block g+4 (shifted layout)."""
    toks = _core_tokens(g)  # 512 global q indices
    m = np.zeros((P, NB, Q), dtype=np.float32)
    for jj in range(NB):
        kk = 128 * jj + np.arange(P)  # global k indices of this tile
        if jj < 8:
            m[:, jj, :] = kk[:, None] <= toks[None, :]
        else:
            m[:, jj, :BLK] = kk[:, None] <= toks[None, BLK:]
    return m.astype(BF16)


def prepare_shared(wq, wk, wv, wo, w_gate, w_up, w_down, ln1_w, ln2_w):
    d = {}
    d["wqt"] = _w5(np.ascontiguousarray(wq.T), KO, KO)
    d["wkt"] = _w5(np.ascontiguousarray(wk.T), KO, KO)
    d["wot"] = _w5(np.ascontiguousarray(wo.T), KO, KO)
    # V projection rhs layout: [128, ko, 2048]
    d["wvt"] = np.ascontiguousarray(
        wv.T.reshape(KO, P, D).transpose(1, 0, 2)
    ).astype(BF16)
    d["wgt"] = _w5(np.ascontiguousarray(w_gate.T), KO, MF)
    d["wut"] = _w5(np.ascontiguousarray(w_up.T), KO, MF)
    d["wdt"] = _w5(np.ascontiguousarray(w_down.T), MF, KO)
    d["ln1"] = np.ascontiguousarray(ln1_w.reshape(KO, P).T).astype(np.float32)
    d["ln2"] = np.ascontiguousarray(ln2_w.reshape(KO, P).T).astype(np.float32)
    return d


def prepare_core(hidden, core: int):
    b, g = core // 4, core % 4
    toks = _core_tokens(g)
    ht = hidden[b][toks].T  # [2048 d, 512 q]
    ht5 = np.ascontiguousarray(ht.reshape(KO, P, Q).transpose(1, 0, 2)).astype(
        np.float32
    )
    return {"ht": ht5, "mask": _core_mask(g)}


def assemble(outs, hidden_dtype):
    """outs: list of per-core [128, KO, 512] fp32 -> full (B, S, D)."""
    full = np.empty((B, S, D), dtype=np.float32)
    for core in range(NCORES):
        b, g = core // 4, core % 4
        toks = _core_tokens(g)
        o = np.asarray(outs[core])  # [p, ko, q]
        full[b, toks, :] = o.transpose(2, 1, 0).reshape(Q, D)
    return full.astype(hidden_dtype)


def _agcol(jj: int) -> int:
    """Global k-tile jj -> row/col offset in the rank-major AG buffers."""
    j = jj // 2  # 256-token block index
    return 512 * (j % 4) + 256 * (j // 4) + 128 * (jj % 2)


# ---------------------------------------------------------------------------
# Pure-numpy simulation of the exact device dataflow (for fast validation)
# ---------------------------------------------------------------------------

def _bf(x):
    return x.astype(BF16).astype(np.float32)


def _sim_norm(ht, lnw):
    # ht: [2048, 512] fp32 (d, q); lnw: [2048]
    sq = _bf(_bf(ht) * _bf(ht))  # DVE squares, bf16 out
    ms = sq.sum(axis=0)  # PE ones-matmul, fp32 accum
    rstd = 1.0 / np.sqrt(ms / D + EPS)
    return _bf((ht * rstd[None, :]) * lnw[:, None])  # bf16 out


def host_simulate(inputs):
    """Numpy replica of the device algorithm, including AG layout and masks."""
    hidden = np.asarray(inputs["hidden_states"], dtype=np.float32)
    f32 = lambda k: np.asarray(inputs[k], dtype=np.float32)  # noqa: E731
    wqT, wkT, wvT, woT = (
        _bf(f32("wq").T), _bf(f32("wk").T), _bf(f32("wv").T), _bf(f32("wo").T)
    )
    wgT, wuT, wdT = _bf(f32("w_gate").T), _bf(f32("w_up").T), _bf(f32("w_down").T)
    ln1, ln2 = f32("ln1_w"), f32("ln2_w")

    kts, vs, xns, hts = {}, {}, {}, {}
    for core in range(NCORES):
        b, g = core // 4, core % 4
        ht = hidden[b][_core_tokens(g)].T  # [2048, 512]
        hts[core] = ht
        xn = _sim_norm(ht, ln1)
        xns[core] = xn
        kts[core] = _bf(wkT.T @ xn)  # kT [2048, 512]
        vs[core] = _bf(xn.T @ wvT)  # v natural [512, 2048]

    outs = []
    for core in range(NCORES):
        b, g = core // 4, core % 4
        grp = [4 * b + r for r in range(4)]
        kt_all = np.concatenate([kts[c] for c in grp], axis=1)  # [2048, 2048]
        v_all = np.concatenate([vs[c] for c in grp], axis=0)  # [2048, 2048]
        mask = np.asarray(_core_mask(g), dtype=np.float32)

        xn = xns[core]
        qT = _bf(wqT.T @ xn)  # [2048, 512]
        attn = np.zeros((D, Q), dtype=np.float32)
        for h in range(H):
            kth = kt_all[h * HD : (h + 1) * HD]  # [128, 2048]
            aps = np.zeros((HD, Q), dtype=np.float32)
            den = np.zeros(Q, dtype=np.float32)
            for jj in range(NB):
                off = _agcol(jj)
                n = Q if jj < 8 else BLK
                sc = kth[:, off : off + P].T @ qT[h * HD : (h + 1) * HD, Q - n :]
                e = _bf(_bf(np.exp(sc * ISQ)) * mask[:, jj, :n])
                vt = v_all[off : off + P, h * HD : (h + 1) * HD]  # [128, hd]
                aps[:, Q - n :] += vt.T @ e
                den[Q - n :] += e.sum(axis=0)
            attn[h * HD : (h + 1) * HD] = _bf(aps * (1.0 / den)[None, :])
        oT = woT.T @ attn  # fp32 accum of bf16 matmul
        h2 = hts[core] + oT
        yT = _sim_norm(h2, ln2)
        gate = wgT.T @ yT
        up = wuT.T @ yT
        sil = _bf(gate / (1.0 + np.exp(-gate)))
        hmid = _bf(sil * up)
        outT = h2 + wdT.T @ hmid
        outs.append(outT.reshape(KO, P, Q).transpose(1, 0, 2).astype(np.float32))
    return assemble(outs, np.asarray(inputs["hidden_states"]).dtype)


# ---------------------------------------------------------------------------
# Device program
# ---------------------------------------------------------------------------

def _build_bass():
    import concourse.bacc as bacc
    import concourse.mybir as mybir
    import concourse.tile as tile

    FP = mybir.dt.float32
    BF = mybir.dt.bfloat16
    AF = mybir.ActivationFunctionType

    nc = bacc.Bacc("TRN2", target_bir_lowering=False, debug=False,
                   num_devices=NCORES)

    ht_d = nc.dram_tensor("ht", [P, KO, Q], FP, kind="ExternalInput")
    mask_d = nc.dram_tensor("mask", [P, NB, Q], BF, kind="ExternalInput")
    ln1_d = nc.dram_tensor("ln1", [P, KO], FP, kind="ExternalInput")
    ln2_d = nc.dram_tensor("ln2", [P, KO], FP, kind="ExternalInput")
    wqt_d = nc.dram_tensor("wqt", [P, KO, KO, P], BF, kind="ExternalInput")
    wkt_d = nc.dram_tensor("wkt", [P, KO, KO, P], BF, kind="ExternalInput")
    wvt_d = nc.dram_tensor("wvt", [P, KO, D], BF, kind="ExternalInput")
    wot_d = nc.dram_tensor("wot", [P, KO, KO, P], BF, kind="ExternalInput")
    wgt_d = nc.dram_tensor("wgt", [P, MF, KO, P], BF, kind="ExternalInput")
    wut_d = nc.dram_tensor("wut", [P, MF, KO, P], BF, kind="ExternalInput")
    wdt_d = nc.dram_tensor("wdt", [P, KO, MF, P], BF, kind="ExternalInput")
    out_d = nc.dram_tensor("out", [P, KO, Q], FP, kind="ExternalOutput")

    groups = [[0, 1, 2, 3], [4, 5, 6, 7]]

    with tile.TileContext(nc) as tc, ExitStack() as top:
        dramp = top.enter_context(tc.tile_pool(name="dram", bufs=1, space="DRAM"))
        constp = top.enter_context(tc.tile_pool(name="const", bufs=1))
        statp = top.enter_context(tc.tile_pool(name="stat", bufs=2))
        workp = top.enter_context(tc.tile_pool(name="work", bufs=3))
        psump = top.enter_context(tc.tile_pool(name="ps", bufs=4, space="PSUM"))
        psaccp = psump

        ones = constp.tile([P, P], BF, tag="ones")
        nc.vector.memset(ones, 1.0)
        eps_t = constp.tile([P, 1], FP, tag="eps")
        nc.vector.memset(eps_t, EPS)
        lnw1 = constp.tile([P, KO], FP, tag="ln1")
        nc.sync.dma_start(lnw1, ln1_d[:])
        lnw2 = constp.tile([P, KO], FP, tag="ln2")
        nc.sync.dma_start(lnw2, ln2_d[:])

        ht_sb = constp.tile([P, KO, Q], FP, tag="ht")  # becomes h2 in place
        nc.sync.dma_start(ht_sb, ht_d[:])
        xn_sb = constp.tile([P, KO, Q], BF, tag="xn")  # x_norm^T, later y^T

        kv_bounce = [dramp.tile([2, D * Q // 2], BF, name=f"kvb{i}")
                     for i in range(2)]
        kv_all = [dramp.tile([8, D * Q // 2], BF, name=f"kva{i}")
                  for i in range(2)]
        kt_b = [b[0:1, :].rearrange("a (d q) -> (a d) q", q=Q)
                for b in kv_bounce]
        v_b = [b[1:2, :].rearrange("a (t d) -> (a t) d", d=D // 2)
               for b in kv_bounce]

        def norm(src, lnw, dst):
            ms = psaccp.tile([P, Q], FP, tag="acc")
            for ko in range(KO):
                sq = workp.tile([P, Q], BF, tag="sq")
                nc.vector.tensor_mul(sq, src[:, ko, :], src[:, ko, :])
                nc.tensor.matmul(ms, ones, sq, start=(ko == 0),
                                 stop=(ko == KO - 1))
            st = statp.tile([P, Q], FP, tag="st")
            nc.scalar.activation(st, ms, AF.Sqrt, bias=eps_t,
                                 scale=1.0 / D)
            rb = statp.tile([P, Q], FP, tag="rb")
            nc.vector.reciprocal(rb, st)
            for ko in range(KO):
                tmp = workp.tile([P, Q], FP, tag="nrm")
                nc.vector.tensor_mul(tmp, src[:, ko, :], rb)
                nc.vector.tensor_scalar_mul(dst[:, ko, :], tmp,
                                            lnw[:, ko : ko + 1])

        # ---- phase 1: norm1 ----
        norm(ht_sb, lnw1, xn_sb)

        with ExitStack() as mid:
            midp = mid.enter_context(tc.tile_pool(name="mid", bufs=1))
            qt_sb = midp.tile([P, KO, Q], BF, tag="qt")
            attn_sb = midp.tile([P, KO, Q], BF, tag="attn")
            with ExitStack() as ctx:
                wp = ctx.enter_context(tc.tile_pool(name="wqkv", bufs=3))
                wvp = ctx.enter_context(tc.tile_pool(name="wvp", bufs=2))

                def proj_t(w5_d, dst_fn, lo=0, hi=KO):
                    for mt in range(lo, hi):
                        wt = wp.tile([P, KO, P], BF, tag="wqk")
                        nc.sync.dma_start(wt, w5_d[:, mt, :, :])
                        ps = psump.tile([P, Q], FP, tag="mm")
                        for ko in range(KO):
                            nc.tensor.matmul(ps, wt[:, ko, :], xn_sb[:, ko, :],
                                             start=(ko == 0),
                                             stop=(ko == KO - 1))
                        dst_fn(mt, ps)

                # ---- phase 2: K projection + AG ----
                def k_out(mt, ps):
                    stg = workp.tile([P, Q], BF, tag="stg")
                    nc.vector.tensor_copy(stg, ps)
                    ml = mt % 8
                    nc.sync.dma_start(
                        kt_b[mt // 8][ml * P : (ml + 1) * P, :], stg
                    )

                # ---- phase 3: V projection, interleaved per-half AGs ----
                def v_chunk(n):
                    wv_c = wvp.tile([P, KO, Q], BF, tag="wv")
                    nc.sync.dma_start(wv_c, wvt_d[:, :, n * Q : (n + 1) * Q])
                    for tt in range(4):
                        ps = psump.tile([P, Q], FP, tag="mm")
                        for ko in range(KO):
                            nc.tensor.matmul(
                                ps, xn_sb[:, ko, tt * P : (tt + 1) * P],
                                wv_c[:, ko, :],
                                start=(ko == 0), stop=(ko == KO - 1),
                            )
                        stg = workp.tile([P, Q], BF, tag="stg")
                        nc.vector.tensor_copy(stg, ps)
                        nc.sync.dma_start(
                            v_b[n // 2][tt * P : (tt + 1) * P,
                                        (n % 2) * Q : (n % 2 + 1) * Q],
                            stg,
                        )

                for hh in range(2):
                    proj_t(wkt_d, k_out, 8 * hh, 8 * hh + 8)
                    v_chunk(2 * hh)
                    v_chunk(2 * hh + 1)
                    nc.gpsimd.collective_compute(
                        "AllGather", mybir.AluOpType.bypass,
                        ins=[kv_bounce[hh].opt()], outs=[kv_all[hh].opt()],
                        replica_groups=groups,
                    )

                # ---- phase 4: Q projection ----
                proj_t(wqt_d, lambda mt, ps:
                       nc.vector.tensor_copy(qt_sb[:, mt, :], ps))

            # ---- phase 5: attention ----
            with ExitStack() as ctx:
                ap = ctx.enter_context(tc.tile_pool(name="attp", bufs=3))
                eap = ctx.enter_context(tc.tile_pool(name="eap", bufs=2))
                maskp = ctx.enter_context(tc.tile_pool(name="maskp", bufs=1))
                mask_sb = maskp.tile([P, NB, Q], BF, tag="mask")
                nc.sync.dma_start(mask_sb, mask_d[:])
                kt_r = [kv_all[hh].rearrange(
                            "(r a) (p q) -> a r p q", a=2, q=Q
                        )[0].rearrange("r (m p) q -> p r m q", p=P)
                        for hh in range(2)]
                v_r = [kv_all[hh].rearrange(
                           "(r a) (t d) -> a r t d", a=2, d=D // 2
                       )[1] for hh in range(2)]

                for h in range(H):
                    hh, hl = h // 8, h % 8
                    kth = ap.tile([P, 4, Q], BF, tag="kth")
                    nc.sync.dma_start(kth, kt_r[hh][:, :, hl, :])
                    kth2 = kth.rearrange("p r q -> p (r q)")
                    vth = ap.tile([P, NB, HD], BF, tag="vth")
                    for jj in range(NB):
                        off = _agcol(jj)
                        nc.sync.dma_start(
                            vth[:, jj, :],
                            v_r[hh][off // Q, off % Q : off % Q + P,
                                    hl * HD : (hl + 1) * HD],
                        )
                    e_all = eap.tile([P, NB, Q], BF, tag="eall")
                    # pass A: scores + exp + mask (needs only KT)
                    for jj in range(NB):
                        n = Q if jj < 8 else BLK
                        off = _agcol(jj)
                        sps = psump.tile([P, Q], FP, tag="mm")
                        nc.tensor.matmul(
                            sps[:, :n], kth2[:, off : off + P],
                            qt_sb[:, h, Q - n :], start=True, stop=True,
                        )
                        nc.scalar.activation(e_all[:, jj, :n], sps[:, :n],
                                             AF.Exp, scale=ISQ)
                        nc.vector.tensor_mul(e_all[:, jj, :n],
                                             e_all[:, jj, :n],
                                             mask_sb[:, jj, :n])
                    # pass B: PV + denominator (needs V)
                    aps = psaccp.tile([P, Q], FP, tag="acc")
                    dps = psaccp.tile([P, Q], FP, tag="acc")
                    for jj in range(NB):
                        n = Q if jj < 8 else BLK
                        osl = slice(Q - n, Q)
                        nc.tensor.matmul(aps[:, osl], vth[:, jj, :],
                                         e_all[:, jj, :n],
                                         start=(jj == 0), stop=(jj == NB - 1))
                        nc.tensor.matmul(dps[:, osl], ones,
                                         e_all[:, jj, :n],
                                         start=(jj == 0), stop=(jj == NB - 1))
                    rec = statp.tile([P, Q], FP, tag="rb")
                    nc.vector.reciprocal(rec, dps)
                    nc.vector.tensor_mul(attn_sb[:, h, :], aps, rec)

            # ---- phase 6: o-projection + residual (into ht_sb) ----
            with ExitStack() as ctx:
                wp = ctx.enter_context(tc.tile_pool(name="wo", bufs=3))
                for mt in range(KO):
                    wt = wp.tile([P, KO, P], BF, tag="wqk")
                    nc.sync.dma_start(wt, wot_d[:, mt, :, :])
                    ps = psump.tile([P, Q], FP, tag="mm")
                    for ko in range(KO):
                        nc.tensor.matmul(ps, wt[:, ko, :], attn_sb[:, ko, :],
                                         start=(ko == 0), stop=(ko == KO - 1))
                    nc.vector.tensor_add(ht_sb[:, mt, :], ps, ht_sb[:, mt, :])

        # ---- phase 7: norm2 (into xn_sb = y^T) ----
        norm(ht_sb, lnw2, xn_sb)

        # ---- phases 8+9: MLP ----
        with ExitStack() as ctx:
            wgp = ctx.enter_context(tc.tile_pool(name="wgu", bufs=2))
            wdp = ctx.enter_context(tc.tile_pool(name="wdp", bufs=2))
            hp = ctx.enter_context(tc.tile_pool(name="hmid", bufs=1))
            hmid = hp.tile([P, MF, Q], BF, tag="hmid")
            for mf in range(MF):
                wg_t = wgp.tile([P, KO, P], BF, tag="wg")
                nc.sync.dma_start(wg_t, wgt_d[:, mf, :, :])
                wu_t = wgp.tile([P, KO, P], BF, tag="wu")
                nc.sync.dma_start(wu_t, wut_d[:, mf, :, :])
                gps = psump.tile([P, Q], FP, tag="mm")
                ups = psump.tile([P, Q], FP, tag="mm")
                for ko in range(KO):
                    nc.tensor.matmul(gps, wg_t[:, ko, :], xn_sb[:, ko, :],
                                     start=(ko == 0), stop=(ko == KO - 1))
                for ko in range(KO):
                    nc.tensor.matmul(ups, wu_t[:, ko, :], xn_sb[:, ko, :],
                                     start=(ko == 0), stop=(ko == KO - 1))
                sil = workp.tile([P, Q], BF, tag="sil")
                nc.scalar.activation(sil, gps, AF.Silu)
                nc.vector.tensor_mul(hmid[:, mf, :], sil, ups)

            for mt in range(KO):
                wd_t = wdp.tile([P, MF, P], BF, tag="wd")
                nc.sync.dma_start(wd_t, wdt_d[:, mt, :, :])
                ps = psump.tile([P, Q], FP, tag="mm")
                for kf in range(MF):
                    nc.tensor.matmul(ps, wd_t[:, kf, :], hmid[:, kf, :],
                                     start=(kf == 0), stop=(kf == MF - 1))
                ot = workp.tile([P, Q], FP, tag="ot")
                nc.vector.tensor_add(ot, ps, ht_sb[:, mt, :])
                nc.sync.dma_start(out_d[:, mt, :], ot)

    nc.compile()
    return nc


_NC_CACHE = None


def kernel(**inputs) -> np.ndarray:
    global _NC_CACHE
    hidden = np.asarray(inputs["hidden_states"])
    shared = prepare_shared(
        np.asarray(inputs["wq"]), np.asarray(inputs["wk"]),
        np.asarray(inputs["wv"]), np.asarray(inputs["wo"]),
        np.asarray(inputs["w_gate"]), np.asarray(inputs["w_up"]),
        np.asarray(inputs["w_down"]), np.asarray(inputs["ln1_w"]),
        np.asarray(inputs["ln2_w"]),
    )
    in_maps = []
    for core in range(NCORES):
        m = dict(shared)
        m.update(prepare_core(np.asarray(hidden, dtype=np.float32), core))
        in_maps.append(m)

    from concourse.bass_utils import run_bass_kernel_spmd

    if _NC_CACHE is None:
        _NC_CACHE = _build_bass()
    nc = _NC_CACHE
    trace = bool(int(os.environ.get("KERNEL_TRACE", "0")))
    res = run_bass_kernel_spmd(
        nc, in_maps, core_ids=list(range(NCORES)), trace=trace
    )
    if trace and res.exec_time_ns is not None:
        print(f"HW exec time: {res.exec_time_ns} ns")
    outs = [res.results[c]["out"] for c in range(NCORES)]
    return assemble(outs, hidden.dtype)
